# revision 2
# baseline (speedup 1.0000x reference)
"""DEP loss (HSIC-style dependence) kernel for Trainium2, 8 NeuronCores.

Math: reference computes sum(K_zm * K_sm) / (norm*n^2) with K_zm/K_sm the
double-centered RBF grams of z and one_hot(s). Because the s-gram is
K_s = e^{-1} + (1-e^{-1})*[s_i==s_j] and double-centering annihilates
constant row/col components, the loss is exactly

    dep = (1-e^{-1})/(norm*n^2) * sum_c  yt_c^T K_z yt_c,   yt_c = y_c - p_c*1

with K_z the *uncentered* z-gram. So the device work is just
G[c,i] = sum_j Y[j,c] * exp(z_j.z_i - |z_j|^2/2 - C)  (C = max|z|^2/2 keeps
exp args <= 0), and a tiny host-side 4x4 reduction finishes the scalar.

Sharding: each core computes G for a 1024-column slab of i, with j running
over all 8192 rows (rows of z broadcast to every core as z^T in bf16).
Per core: 64 j-tiles x [128 x 1024]: PE gram matmul -> ACT exp(+bias) ->
PE one-hot reduction matmul accumulating G in PSUM.
"""

import os
import numpy as np
import ml_dtypes
from contextlib import ExitStack

N = 8192
D = 128
NCLS = 4
NCORES = 8
SLAB = N // NCORES  # 1024 i-columns per core
JT = N // 128       # 64 j-tiles
NH = SLAB // 512    # PSUM-width halves per slab

_NC_CACHE = {}


def _build_nc(reps=1):
    import concourse.bacc as bacc
    import concourse.tile as tile
    from concourse import mybir

    nc = bacc.Bacc(
        "TRN2", target_bir_lowering=False, debug=False, num_devices=NCORES
    )
    bf16 = mybir.dt.bfloat16
    f32 = mybir.dt.float32

    zt = nc.dram_tensor("zt", [128, N], bf16, kind="ExternalInput").ap()
    zs = nc.dram_tensor("zs", [128, SLAB], bf16, kind="ExternalInput").ap()
    yp = nc.dram_tensor("yp", [128, JT * NCLS], bf16, kind="ExternalInput").ap()
    bj = nc.dram_tensor("bj", [128, JT], f32, kind="ExternalInput").ap()
    g = nc.dram_tensor("g", [NCLS, SLAB], f32, kind="ExternalOutput").ap()

    with tile.TileContext(nc) as tc, ExitStack() as ctx:
        const = ctx.enter_context(tc.tile_pool(name="const", bufs=1))
        psum_t = ctx.enter_context(tc.tile_pool(name="psumt", bufs=4, space="PSUM"))
        psum_g = ctx.enter_context(tc.tile_pool(name="psumg", bufs=1, space="PSUM"))
        tpool = ctx.enter_context(tc.tile_pool(name="texp", bufs=4))
        gpool = ctx.enter_context(tc.tile_pool(name="gsb", bufs=1))

        zt_sb = const.tile([128, N], bf16, tag="zt")
        for k in range(8):
            nc.sync.dma_start(
                out=zt_sb[:, k * 1024 : (k + 1) * 1024],
                in_=zt[:, k * 1024 : (k + 1) * 1024],
            )
        zs_sb = const.tile([128, SLAB], bf16, tag="zs")
        for k in range(NH):
            nc.sync.dma_start(
                out=zs_sb[:, k * 512 : (k + 1) * 512],
                in_=zs[:, k * 512 : (k + 1) * 512],
            )
        yp_sb = const.tile([128, JT * NCLS], bf16, tag="yp")
        nc.sync.dma_start(out=yp_sb[:], in_=yp[:])
        bj_sb = const.tile([128, JT], f32, tag="bj")
        nc.sync.dma_start(out=bj_sb[:], in_=bj[:])

        gps = [
            psum_g.tile([NCLS, 512], f32, tag=f"g{h}", name=f"gps{h}")
            for h in range(NH)
        ]

        for rep in range(reps):
            for jt in range(JT):
                lhsT = zt_sb[:, jt * 128 : (jt + 1) * 128]
                yslc = yp_sb[:, jt * NCLS : (jt + 1) * NCLS]
                bslc = bj_sb[:, jt : jt + 1]
                for h in range(NH):
                    pt = psum_t.tile([128, 512], f32, tag="pt")
                    nc.tensor.matmul(
                        pt[:],
                        lhsT,
                        zs_sb[:, h * 512 : (h + 1) * 512],
                        start=True,
                        stop=True,
                    )
                    tt = tpool.tile([128, 512], bf16, tag="tt")
                    nc.scalar.activation(
                        tt[:],
                        pt[:],
                        mybir.ActivationFunctionType.Exp,
                        bias=bslc,
                        scale=1.0,
                    )
                    nc.tensor.matmul(
                        gps[h][:],
                        yslc,
                        tt[:],
                        start=(jt == 0),
                        stop=(jt == JT - 1),
                    )

        g_sb = gpool.tile([NCLS, SLAB], f32, tag="gsb")
        for h in range(NH):
            nc.vector.tensor_copy(g_sb[:, h * 512 : (h + 1) * 512], gps[h][:])
        nc.sync.dma_start(out=g[:], in_=g_sb[:])

    nc.compile()
    return nc


def _get_nc(reps=1):
    if reps not in _NC_CACHE:
        _NC_CACHE[reps] = _build_nc(reps)
    return _NC_CACHE[reps]


def _prep_inputs(z, s):
    zb = np.asarray(z, dtype=np.float32).astype(ml_dtypes.bfloat16)
    zt_np = np.ascontiguousarray(zb.T)  # [128, N]
    zf = zb.astype(np.float64)
    sq = (zf * zf).sum(1)  # [N]
    C = sq.max() / 2.0
    bias = (-sq / 2.0 - C).astype(np.float32)
    bj_np = np.ascontiguousarray(bias.reshape(JT, 128).T)  # [128, JT]
    s_i = np.asarray(s).astype(np.int64)
    Y = s_i[:, None] == np.arange(NCLS, dtype=np.int64)[None, :]  # [N, 4] bool
    yp_np = np.ascontiguousarray(
        Y.reshape(JT, 128, NCLS).transpose(1, 0, 2).reshape(128, JT * NCLS)
    ).astype(ml_dtypes.bfloat16)
    return zt_np, bj_np, yp_np, Y, sq, C


def run_device(z, s, reps=1):
    """Run the SPMD device kernel; returns raw per-core G [4, N] (float64) plus
    the host-side rescale vector pieces."""
    from concourse.bass_utils import run_bass_kernel_spmd

    zt_np, bj_np, yp_np, Y, sq, C = _prep_inputs(z, s)
    in_maps = []
    for c in range(NCORES):
        in_maps.append(
            {
                "zt": zt_np,
                "zs": np.ascontiguousarray(zt_np[:, c * SLAB : (c + 1) * SLAB]),
                "yp": yp_np,
                "bj": bj_np,
            }
        )
    nc = _get_nc(reps)
    res = run_bass_kernel_spmd(nc, in_maps, list(range(NCORES))).results
    G = np.concatenate([res[c]["g"] for c in range(NCORES)], axis=1).astype(
        np.float64
    )  # [4, N], G[c_class, i] = sum_j Y[j,c] exp(zz - sqj/2 - C)
    return G, Y, sq, C


def _finish(G, Y, sq, C, norm_v):
    G = G * np.exp(C - sq / 2.0)[None, :]  # true G[c, i]
    Yf = Y.astype(np.float64)
    A = Yf.T @ G.T  # A[a,b] = sum_i Y[i,a] G[b,i]
    p = Yf.mean(0)
    S = A.sum()
    rows = A.sum(1)
    cols = A.sum(0)
    acc = sum(
        A[c, c] - p[c] * rows[c] - p[c] * cols[c] + p[c] ** 2 * S
        for c in range(NCLS)
    )
    dep = (1.0 - np.exp(-1.0)) * acc / (norm_v * N * N)
    return np.array(dep, dtype=np.float32)


def kernel(z, s, norm):
    norm_v = float(np.asarray(norm))
    G, Y, sq, C = run_device(z, s, reps=1)
    return _finish(G, Y, sq, C, norm_v)


if __name__ == "__main__":
    rng = np.random.default_rng(0)
    z = rng.standard_normal((N, D), dtype=np.float32)
    s = rng.integers(0, NCLS, size=(N,)).astype(np.int64)
    print(kernel(z, s, np.float32(1.0)))


# revision 4
# speedup vs baseline: 1.2144x; 1.2144x over previous
"""DEP loss (HSIC-style dependence) kernel for Trainium2, 8 NeuronCores.

Math: reference computes sum(K_zm * K_sm) / (norm*n^2) with K_zm/K_sm the
double-centered RBF grams of z and one_hot(s). Because the s-gram is
K_s = e^{-1} + (1-e^{-1})*[s_i==s_j] and double-centering annihilates
constant row/col components, the loss is exactly

    dep = (1-e^{-1})/(norm*n^2) * sum_c  yt_c^T K_z yt_c,   yt_c = y_c - p_c*1

with K_z the *uncentered* z-gram. So the device work is just
G[c,i] = sum_j Y[j,c] * exp(z_j.z_i - |z_j|^2/2 - C)  (C = max|z|^2/2 keeps
exp args <= 0), and a tiny host-side 4x4 reduction finishes the scalar.

Sharding: each core computes G for a 1024-column slab of i, with j running
over all 8192 rows (rows of z broadcast to every core as z^T in bf16).
Per core: 64 j-tiles x [128 x 1024]: PE gram matmul -> ACT exp(+bias) ->
PE one-hot reduction matmul accumulating G in PSUM.
"""

import os
import numpy as np
import ml_dtypes
from contextlib import ExitStack

N = 8192
D = 128
NCLS = 4
NCORES = 8
SLAB = N // NCORES  # 1024 i-columns per core
JT = N // 128       # 64 j-tiles
NH = SLAB // 512    # PSUM-width halves per slab

_NC_CACHE = {}


def _build_nc(reps=1):
    import concourse.bacc as bacc
    import concourse.tile as tile
    from concourse import mybir

    nc = bacc.Bacc(
        "TRN2", target_bir_lowering=False, debug=False, num_devices=NCORES
    )
    bf16 = mybir.dt.bfloat16
    f32 = mybir.dt.float32

    zt = nc.dram_tensor("zt", [128, N], bf16, kind="ExternalInput").ap()
    zs = nc.dram_tensor("zs", [128, SLAB], bf16, kind="ExternalInput").ap()
    yp = nc.dram_tensor("yp", [128, JT * NCLS], bf16, kind="ExternalInput").ap()
    bj = nc.dram_tensor("bj", [128, JT], f32, kind="ExternalInput").ap()
    g = nc.dram_tensor("g", [NCLS, SLAB], f32, kind="ExternalOutput").ap()

    with tile.TileContext(nc) as tc, ExitStack() as ctx:
        const = ctx.enter_context(tc.tile_pool(name="const", bufs=1))
        psum_t = ctx.enter_context(tc.tile_pool(name="psumt", bufs=3, space="PSUM"))
        psum_g = ctx.enter_context(tc.tile_pool(name="psumg", bufs=1, space="PSUM"))
        tpool = ctx.enter_context(tc.tile_pool(name="texp", bufs=3))
        gpool = ctx.enter_context(tc.tile_pool(name="gsb", bufs=1))

        zt_sb = const.tile([128, N], bf16, tag="zt")
        for k in range(8):
            nc.sync.dma_start(
                out=zt_sb[:, k * 1024 : (k + 1) * 1024],
                in_=zt[:, k * 1024 : (k + 1) * 1024],
            )
        zs_sb = const.tile([128, SLAB], bf16, tag="zs")
        for k in range(NH):
            nc.sync.dma_start(
                out=zs_sb[:, k * 512 : (k + 1) * 512],
                in_=zs[:, k * 512 : (k + 1) * 512],
            )
        yp_sb = const.tile([128, JT * NCLS], bf16, tag="yp")
        nc.sync.dma_start(out=yp_sb[:], in_=yp[:])
        bj_sb = const.tile([128, JT], f32, tag="bj")
        nc.sync.dma_start(out=bj_sb[:], in_=bj[:])

        gps = [
            psum_g.tile([NCLS, 512], f32, tag=f"g{h}", name=f"gps{h}")
            for h in range(NH)
        ]

        for rep in range(reps):
            for jt in range(JT):
                lhsT = zt_sb[:, jt * 128 : (jt + 1) * 128]
                yslc = yp_sb[:, jt * NCLS : (jt + 1) * NCLS]
                bslc = bj_sb[:, jt : jt + 1]
                # [128, 1024] PSUM tile spanning both i-halves: two matmuls
                # (one per bank), ONE wide ACT to amortize the ScalarE
                # PSUM-source bubble, then two reduce matmuls.
                pt = psum_t.tile([128, SLAB], f32, tag="pt", name=f"pt_{rep}_{jt}")
                for h in range(NH):
                    nc.tensor.matmul(
                        pt[:, h * 512 : (h + 1) * 512],
                        lhsT,
                        zs_sb[:, h * 512 : (h + 1) * 512],
                        start=True,
                        stop=True,
                    )
                tt = tpool.tile([128, SLAB], bf16, tag="tt", name=f"tt_{rep}_{jt}")
                nc.scalar.activation(
                    tt[:],
                    pt[:],
                    mybir.ActivationFunctionType.Exp,
                    bias=bslc,
                    scale=1.0,
                )
                for h in range(NH):
                    nc.tensor.matmul(
                        gps[h][:],
                        yslc,
                        tt[:, h * 512 : (h + 1) * 512],
                        start=(jt == 0),
                        stop=(jt == JT - 1),
                    )

        g_sb = gpool.tile([NCLS, SLAB], f32, tag="gsb")
        for h in range(NH):
            nc.vector.tensor_copy(g_sb[:, h * 512 : (h + 1) * 512], gps[h][:])
        nc.sync.dma_start(out=g[:], in_=g_sb[:])

    nc.compile()
    return nc


def _get_nc(reps=1):
    if reps not in _NC_CACHE:
        _NC_CACHE[reps] = _build_nc(reps)
    return _NC_CACHE[reps]


def _prep_inputs(z, s):
    zb = np.asarray(z, dtype=np.float32).astype(ml_dtypes.bfloat16)
    zt_np = np.ascontiguousarray(zb.T)  # [128, N]
    zf = zb.astype(np.float64)
    sq = (zf * zf).sum(1)  # [N]
    C = sq.max() / 2.0
    bias = (-sq / 2.0 - C).astype(np.float32)
    bj_np = np.ascontiguousarray(bias.reshape(JT, 128).T)  # [128, JT]
    s_i = np.asarray(s).astype(np.int64)
    Y = s_i[:, None] == np.arange(NCLS, dtype=np.int64)[None, :]  # [N, 4] bool
    yp_np = np.ascontiguousarray(
        Y.reshape(JT, 128, NCLS).transpose(1, 0, 2).reshape(128, JT * NCLS)
    ).astype(ml_dtypes.bfloat16)
    return zt_np, bj_np, yp_np, Y, sq, C


def run_device(z, s, reps=1):
    """Run the SPMD device kernel; returns raw per-core G [4, N] (float64) plus
    the host-side rescale vector pieces."""
    from concourse.bass_utils import run_bass_kernel_spmd

    zt_np, bj_np, yp_np, Y, sq, C = _prep_inputs(z, s)
    in_maps = []
    for c in range(NCORES):
        in_maps.append(
            {
                "zt": zt_np,
                "zs": np.ascontiguousarray(zt_np[:, c * SLAB : (c + 1) * SLAB]),
                "yp": yp_np,
                "bj": bj_np,
            }
        )
    nc = _get_nc(reps)
    res = run_bass_kernel_spmd(nc, in_maps, list(range(NCORES))).results
    G = np.concatenate([res[c]["g"] for c in range(NCORES)], axis=1).astype(
        np.float64
    )  # [4, N], G[c_class, i] = sum_j Y[j,c] exp(zz - sqj/2 - C)
    return G, Y, sq, C


def _finish(G, Y, sq, C, norm_v):
    G = G * np.exp(C - sq / 2.0)[None, :]  # true G[c, i]
    Yf = Y.astype(np.float64)
    A = Yf.T @ G.T  # A[a,b] = sum_i Y[i,a] G[b,i]
    p = Yf.mean(0)
    S = A.sum()
    rows = A.sum(1)
    cols = A.sum(0)
    acc = sum(
        A[c, c] - p[c] * rows[c] - p[c] * cols[c] + p[c] ** 2 * S
        for c in range(NCLS)
    )
    dep = (1.0 - np.exp(-1.0)) * acc / (norm_v * N * N)
    return np.array(dep, dtype=np.float32)


def kernel(z, s, norm):
    norm_v = float(np.asarray(norm))
    G, Y, sq, C = run_device(z, s, reps=1)
    return _finish(G, Y, sq, C, norm_v)


if __name__ == "__main__":
    rng = np.random.default_rng(0)
    z = rng.standard_normal((N, D), dtype=np.float32)
    s = rng.integers(0, NCLS, size=(N,)).astype(np.int64)
    print(kernel(z, s, np.float32(1.0)))


# revision 5
# speedup vs baseline: 1.5708x; 1.2935x over previous
"""DEP loss (HSIC-style dependence) kernel for Trainium2, 8 NeuronCores.

Math: reference computes sum(K_zm * K_sm) / (norm*n^2) with K_zm/K_sm the
double-centered RBF grams of z and one_hot(s). Because the s-gram is
K_s = e^{-1} + (1-e^{-1})*[s_i==s_j] and double-centering annihilates
constant row/col components, the loss is exactly

    dep = (1-e^{-1})/(norm*n^2) * sum_c  yt_c^T K_z yt_c,   yt_c = y_c - p_c*1

with K_z the *uncentered* z-gram. So the device work is just
G[c,i] = sum_j Y[j,c] * exp(z_j.z_i - |z_j|^2/2 - C)  (C = max|z|^2/2 keeps
exp args <= 0), and a tiny host-side 4x4 reduction finishes the scalar.

Sharding: each core computes G for a 1024-column slab of i, with j running
over all 8192 rows (rows of z broadcast to every core as z^T in bf16).
Per core: 64 j-tiles x [128 x 1024]: PE gram matmul -> ACT exp(+bias) ->
PE one-hot reduction matmul accumulating G in PSUM.
"""

import numpy as np
import ml_dtypes
from contextlib import ExitStack

N = 8192
D = 128
NCLS = 4
NCORES = 8
SLAB = N // NCORES  # 1024 i-columns per core
JT = N // 128       # 64 j-tiles
NH = SLAB // 512    # PSUM-width halves per slab

_NC_CACHE = {}


def _build_nc(reps=1):
    import concourse.bacc as bacc
    import concourse.tile as tile
    from concourse import mybir

    nc = bacc.Bacc(
        "TRN2", target_bir_lowering=False, debug=False, num_devices=NCORES
    )
    bf16 = mybir.dt.bfloat16
    f32 = mybir.dt.float32

    zt = nc.dram_tensor("zt", [128, N], bf16, kind="ExternalInput").ap()
    zs = nc.dram_tensor("zs", [128, SLAB], bf16, kind="ExternalInput").ap()
    yp = nc.dram_tensor("yp", [128, JT * NCLS], bf16, kind="ExternalInput").ap()
    bj = nc.dram_tensor("bj", [128, JT], f32, kind="ExternalInput").ap()
    g = nc.dram_tensor("g", [NCLS, SLAB], f32, kind="ExternalOutput").ap()

    with tile.TileContext(nc) as tc, ExitStack() as ctx:
        const = ctx.enter_context(tc.tile_pool(name="const", bufs=1))
        psum_t = ctx.enter_context(tc.tile_pool(name="psumt", bufs=3, space="PSUM"))
        psum_g = ctx.enter_context(tc.tile_pool(name="psumg", bufs=1, space="PSUM"))
        tpool = ctx.enter_context(tc.tile_pool(name="texp", bufs=3))
        gpool = ctx.enter_context(tc.tile_pool(name="gsb", bufs=1))

        zt_sb = const.tile([128, N], bf16, tag="zt")
        for k in range(8):
            nc.sync.dma_start(
                out=zt_sb[:, k * 1024 : (k + 1) * 1024],
                in_=zt[:, k * 1024 : (k + 1) * 1024],
            )
        zs_sb = const.tile([128, SLAB], bf16, tag="zs")
        for k in range(NH):
            nc.sync.dma_start(
                out=zs_sb[:, k * 512 : (k + 1) * 512],
                in_=zs[:, k * 512 : (k + 1) * 512],
            )
        yp_sb = const.tile([128, JT * NCLS], bf16, tag="yp")
        nc.sync.dma_start(out=yp_sb[:], in_=yp[:])
        bj_sb = const.tile([128, JT], f32, tag="bj")
        nc.sync.dma_start(out=bj_sb[:], in_=bj[:])

        gps = [
            psum_g.tile([NCLS, 512], f32, tag=f"g{h}", name=f"gps{h}")
            for h in range(NH)
        ]

        for rep in range(reps):
            for jt in range(JT):
                lhsT = zt_sb[:, jt * 128 : (jt + 1) * 128]
                yslc = yp_sb[:, jt * NCLS : (jt + 1) * NCLS]
                bslc = bj_sb[:, jt : jt + 1]
                # [128, 1024] PSUM tile spanning both i-halves: two matmuls
                # (one per bank), ONE wide ACT to amortize the ScalarE
                # PSUM-source bubble, then two reduce matmuls.
                pt = psum_t.tile([128, SLAB], f32, tag="pt", name=f"pt_{rep}_{jt}")
                for h in range(NH):
                    nc.tensor.matmul(
                        pt[:, h * 512 : (h + 1) * 512],
                        lhsT,
                        zs_sb[:, h * 512 : (h + 1) * 512],
                        start=True,
                        stop=True,
                    )
                tt = tpool.tile([128, SLAB], bf16, tag="tt", name=f"tt_{rep}_{jt}")
                nc.scalar.activation(
                    tt[:],
                    pt[:],
                    mybir.ActivationFunctionType.Exp,
                    bias=bslc,
                    scale=1.0,
                )
                for h in range(NH):
                    nc.tensor.matmul(
                        gps[h][:],
                        yslc,
                        tt[:, h * 512 : (h + 1) * 512],
                        start=(jt == 0),
                        stop=(jt == JT - 1),
                    )

        g_sb = gpool.tile([NCLS, SLAB], f32, tag="gsb")
        for h in range(NH):
            nc.vector.tensor_copy(g_sb[:, h * 512 : (h + 1) * 512], gps[h][:])
        nc.sync.dma_start(out=g[:], in_=g_sb[:])

    nc.compile()
    return nc


def _get_nc(reps=1):
    if reps not in _NC_CACHE:
        _NC_CACHE[reps] = _build_nc(reps)
    return _NC_CACHE[reps]


def _prep_inputs(z, s):
    zb = np.asarray(z, dtype=np.float32).astype(ml_dtypes.bfloat16)
    zt_np = np.ascontiguousarray(zb.T)  # [128, N]
    zf = zb.astype(np.float64)
    sq = (zf * zf).sum(1)  # [N]
    C = sq.max() / 2.0
    bias = (-sq / 2.0 - C).astype(np.float32)
    bj_np = np.ascontiguousarray(bias.reshape(JT, 128).T)  # [128, JT]
    s_i = np.asarray(s).astype(np.int64)
    Y = s_i[:, None] == np.arange(NCLS, dtype=np.int64)[None, :]  # [N, 4] bool
    yp_np = np.ascontiguousarray(
        Y.reshape(JT, 128, NCLS).transpose(1, 0, 2).reshape(128, JT * NCLS)
    ).astype(ml_dtypes.bfloat16)
    return zt_np, bj_np, yp_np, Y, sq, C


def run_device(z, s, reps=1):
    """Run the SPMD device kernel; returns raw per-core G [4, N] (float64) plus
    the host-side rescale vector pieces."""
    from concourse.bass_utils import run_bass_kernel_spmd

    zt_np, bj_np, yp_np, Y, sq, C = _prep_inputs(z, s)
    in_maps = []
    for c in range(NCORES):
        in_maps.append(
            {
                "zt": zt_np,
                "zs": np.ascontiguousarray(zt_np[:, c * SLAB : (c + 1) * SLAB]),
                "yp": yp_np,
                "bj": bj_np,
            }
        )
    nc = _get_nc(reps)
    res = run_bass_kernel_spmd(nc, in_maps, list(range(NCORES))).results
    G = np.concatenate([res[c]["g"] for c in range(NCORES)], axis=1).astype(
        np.float64
    )  # [4, N], G[c_class, i] = sum_j Y[j,c] exp(zz - sqj/2 - C)
    return G, Y, sq, C


def _finish(G, Y, sq, C, norm_v):
    G = G * np.exp(C - sq / 2.0)[None, :]  # true G[c, i]
    Yf = Y.astype(np.float64)
    A = Yf.T @ G.T  # A[a,b] = sum_i Y[i,a] G[b,i]
    p = Yf.mean(0)
    S = A.sum()
    rows = A.sum(1)
    cols = A.sum(0)
    acc = sum(
        A[c, c] - p[c] * rows[c] - p[c] * cols[c] + p[c] ** 2 * S
        for c in range(NCLS)
    )
    dep = (1.0 - np.exp(-1.0)) * acc / (norm_v * N * N)
    return np.array(dep, dtype=np.float32)


def kernel(z, s, norm):
    norm_v = float(np.asarray(norm))
    G, Y, sq, C = run_device(z, s, reps=1)
    return _finish(G, Y, sq, C, norm_v)


if __name__ == "__main__":
    rng = np.random.default_rng(0)
    z = rng.standard_normal((N, D), dtype=np.float32)
    s = rng.integers(0, NCLS, size=(N,)).astype(np.int64)
    print(kernel(z, s, np.float32(1.0)))
